# revision 16
# baseline (speedup 1.0000x reference)
"""TRN2 Bass kernel for nn_Encoder (two-phase LSTM over huge batch).

Self-contained: takes the FULL unsharded inputs, shards the batch across
8 NeuronCores (pure data parallel), runs a Bass/Tile kernel per core via
run_bass_kernel_spmd, and reassembles the full outputs.

Device layout (per core, batch B_c = 65536):
  - batch split into 8 chains of 16*512; slice s=0..15 covers 512 columns
    of a chain; SBUF partition p = 8*s + r  <->  (slice s, feature r).
  - one fp16 matmul per gate bank per step: M=128, K=128, block-diagonal
    lhsT (16 8x8 blocks); PSUM accumulates fp32 (x-side + h-side).
  - bank order [F, I, O, G2]; the G rows of the weights are pre-scaled
    by 2 on the host so that tanh(g) = 2*sigmoid(2g) - 1 and ONE Sigmoid
    ACTIVATE covers all four banks (FD=2048) - the scalar engine is the
    eval-count bottleneck, so every saved instruction/eval matters.
  - cell state is stored pre-scaled (ch = c / A): the custom DVE op
    TANH5_ANT (clamp to [-1,1] + odd deg-5 poly, 8 ALU stages, one DVE
    instruction at 1 elem/cycle) evaluates tanh(A*ch) without needing an
    input-scale stage.  tanh work thus moves off the scalar engine for
    all non-output steps; output steps (t=7, 19) use the exact ACT tanh
    (activation scale=A) so the emitted h has table accuracy.
  - DVE (chain-major group tiles, ops batched over a whole group so the
    per-instruction overhead amortizes and packed 2x/4x modes engage):
    gp=(G2-0.5)*(2/A) [tensor_scalar 4x], (u|v)=(F|I)*(ch|gp) in one
    tensor_tensor [2x], ch=u+v [2x], t=TANH5(ch) [1x], h=O*t [2x].
  - x-tiles pack 3 timesteps (row 2*tau+k = x[t0+tau][k]) plus a ones row
    carrying the fused bias; input embedding + biases folded into lhsT;
    x chunks prefetched one chunk ahead on the sync DMA queue.
  - the 8 chains run as 3 groups (3, 3, 2): three pipeline streams whose
    per-group serial path (MMs -> sigmoid -> DVE chain) hides under the
    engine-bound period; PSUM rotates 2 four-bank slots across chains.

  Engine balance per chain-step (512 cols x 16 slices): PE 8 matmuls
  ~1.7us, ACT one FD=2048 sigmoid ~1.9us, DVE ~1.8us - triple-balanced;
  measured 352.6us vs 577.6us baseline, rel err 1.27e-3 (gate 2e-2).
"""

import os
import sys

for _p in ("/opt/trn_rl_repo", "/root/.axon_site/_ro/trn_rl_repo"):
    if os.path.isdir(_p) and _p not in sys.path:
        sys.path.insert(0, _p)
        break

import numpy as np

import concourse.bacc as bacc
import concourse.mybir as mybir
import concourse.tile as tile
from concourse import bass_utils

F32 = mybir.dt.float32
F16 = mybir.dt.float16
AF = mybir.ActivationFunctionType
ALU = mybir.AluOpType

B = 524288
N_CORES = 8
B_C = B // N_CORES
N = 512
SLICES = 16
PASS = SLICES * N
N_PASS = B_C // PASS
T_OBS, T_PRE, IN, H = 8, 12, 2, 8
XPACK = 3
N_CHUNK_OBS = (T_OBS + XPACK - 1) // XPACK
N_CHUNK_PRE = (T_PRE + XPACK - 1) // XPACK
N_CHAINS = 8
# bank order: F, I, O, G (pytorch gate order in the weight rows is
# i, f, g, o).  G rows are scaled x2 (sigmoid trick).
BANK_GATE = [1, 0, 3, 2]
G_BANK = 3

# tanh(A*v) ~= v*(C0 + v^2*(C1 + v^2*C2)) on v in [-1, 1] (distribution-
# weighted fit over the cell-state values this model actually produces).
A_SCALE = 1.5
T5_C0, T5_C1, T5_C2 = 1.48754895, -0.93932143, 0.3667481


# ------------------------------------------------- custom DVE op (TANH5)

_TANH5 = [None]


def _tanh5_ref(in0, in1, s0, s1, imm2):
    v = np.clip(in0, -1.0, 1.0)
    u = v * v
    return v * (s0 + u * (s1 + u * imm2))


def _get_tanh5():
    if _TANH5[0] is not None:
        return _TANH5[0]
    import concourse.dve_ops as dve_ops
    for op in dve_ops.OPS:
        if op.name == "TANH5_ANT":
            _TANH5[0] = op
            return op
    from concourse.dve_spec import Spec, Src0, C0, C1, C2, Zero, One, \
        maxx, minn, sq
    v = maxx(minn(Src0, One), Zero - One)
    u = sq(v)
    spec = Spec(body=v * (C0 + u * (C1 + u * C2)), reference=_tanh5_ref)
    op = dve_ops.DveOp(
        "TANH5_ANT", spec, subdim=False,
        uops_sha={"v3": "71a28649e0d00322", "v4": "c2c1420f4c6160a6"})
    dve_ops.OPS.append(op)
    dve_ops.CUSTOM_DVE_SPECS[op.name] = spec
    dve_ops._SUB_OPCODE_FOR_NAME[op.name] = (
        dve_ops._CUSTOM_DVE_ROW_BASE + len(dve_ops.OPS) - 1)
    assert dve_ops._SUB_OPCODE_FOR_NAME[op.name] < 0x20
    _TANH5[0] = op
    return op


# ---------------------------------------------------------------- host prep

def _make_weights(W_in, b_in, W_ih, W_hh, b_ih, b_hh):
    """lhsT arrays: w_gx [XPACK, 128, 4, 128] (tau,p,bank,m), w_gh [128,4,128].

    Block-diagonal over the 16 slices: one M=128, K=128 matmul per gate bank
    computes that bank for all 16 slices at once.  The G bank rows carry 2x
    the weights/bias (sigmoid trick).
    """
    Wx = (W_ih @ W_in).astype(np.float32)
    bias = (W_ih @ b_in + b_ih + b_hh).astype(np.float32)
    w_gx = np.zeros((XPACK, 128, 4, 128), np.float32)
    w_gh = np.zeros((128, 4, 128), np.float32)
    for b in range(4):
        g = BANK_GATE[b]
        sc = 2.0 if b == G_BANK else 1.0
        for s in range(16):
            for r in range(H):
                col = 8 * s + r
                for tau in range(XPACK):
                    for k in range(IN):
                        w_gx[tau, 8 * s + 2 * tau + k, b, col] = \
                            sc * Wx[g * H + r, k]
                    w_gx[tau, 8 * s + 6, b, col] = sc * bias[g * H + r]
                w_gh[8 * s: 8 * s + H, b, col] = sc * W_hh[g * H + r, :]
    return w_gx.astype(np.float16), w_gh.astype(np.float16)


def _shuffle_state(aT):
    """[8, B_c] -> [N_PASS, 128, N] device layout (p, 8s+r, n)."""
    return np.ascontiguousarray(
        aT.reshape(H, N_PASS, SLICES, N).transpose(1, 2, 0, 3).reshape(
            N_PASS, 128, N).astype(np.float16))


def _unshuffle_state(dev):
    """[N_PASS, 128, N] -> [8, B_c]."""
    return dev.reshape(N_PASS, SLICES, H, N).transpose(2, 0, 1, 3).reshape(
        H, B_C)


def _pack_x(x):
    """[T, 2, B_c] -> [n_chunk, N_PASS, 128, N]: 3 steps + ones row baked."""
    T = x.shape[0]
    n_chunk = (T + XPACK - 1) // XPACK
    out = np.zeros((n_chunk, N_PASS, SLICES, 8, N), np.float32)
    out[:, :, :, 6, :] = 1.0
    for tau in range(XPACK):
        for k in range(IN):
            for t3 in range(n_chunk):
                t = t3 * XPACK + tau
                if t < T:
                    out[t3, :, :, 2 * tau + k, :] = x[t, k].reshape(
                        N_PASS, SLICES, N)
    return np.ascontiguousarray(
        out.reshape(n_chunk, N_PASS, 128, N).astype(np.float16))


def _prep_core_inputs(inputs, lo, hi, weights):
    g = lambda k: np.asarray(inputs[k], np.float32)
    d = {}
    d["x_obs"] = _pack_x(
        np.ascontiguousarray(g("obs_traj_rel")[:, lo:hi, :].transpose(0, 2, 1)))
    d["x_pre"] = _pack_x(
        np.ascontiguousarray(g("pre_traj_rel")[:, lo:hi, :].transpose(0, 2, 1)))
    d["hT0"] = _shuffle_state(np.ascontiguousarray(g("h0")[lo:hi].T))
    d["cT0"] = _shuffle_state(
        np.ascontiguousarray(g("c0")[lo:hi].T) / A_SCALE)
    d["cT0_pre"] = _shuffle_state(
        np.ascontiguousarray(g("c0_pre")[lo:hi].T) / A_SCALE)
    d.update(weights)
    return d


# ------------------------------------------------------------- device build

GROUPS = (3, 3, 2)  # chains per DVE-shared tile group (3 pipeline streams)


def _build_kernel(tc, outs, ins):
    nc = tc.nc
    tanh5 = _get_tanh5()
    state = tc.alloc_tile_pool(name="state", bufs=1)
    psump = tc.alloc_tile_pool(name="psum", bufs=2, space="PSUM")

    # obs-phase weights load first (they gate the very first matmul);
    # pre-phase weights are deferred until after the initial state loads
    # (not needed until t = T_OBS).
    wsb = {}
    for key in ("w_gx_obs", "w_gx_pre"):
        wsb[key] = state.tile([128, XPACK, 4, 128], F16, name=key + "_sb",
                              tag=key)
    for key in ("w_gh_obs", "w_gh_pre"):
        wsb[key] = state.tile([128, 4, 128], F16, name=key + "_sb", tag=key)
    # first chain's state loads are emitted in the group-init loop below;
    # obs weights follow immediately after the first chain's tiles there.
    _w_obs_loaded = [False]

    def _load_obs_weights():
        if _w_obs_loaded[0]:
            return
        _w_obs_loaded[0] = True
        nc.sync.dma_start(wsb["w_gx_obs"],
                          ins["w_gx_obs"].rearrange("t p b m -> p t b m"))
        nc.sync.dma_start(wsb["w_gh_obs"], ins["w_gh_obs"])

    # Chains in a group share one set of wide tiles so every DVE
    # instruction covers group*512 elements (per-instruction overhead
    # amortized, all ops in 2x/4x packed modes).  Three groups give three
    # pipeline streams so the per-group serial path (MM -> sigmoid ->
    # DVE chain) hides under the engine-bound period.  Layouts:
    #   T   [g chains][4 banks F,I,O,G2][N]  <- contiguous-out sigmoids
    #   cgp [g chains][2 slots ch,gp   ][N]
    #   uv  [g chains][2 slots u,v     ][N]
    #   t,h [g chains][N]
    quads = []
    for qi, gsz in enumerate(GROUPS):
        q = {
            "n": gsz,
            "T": state.tile([128, gsz, 4, N], F16, name=f"T_{qi}",
                            tag=f"T_{qi}"),
            "cgp": state.tile([128, gsz, 2, N], F16, name=f"cgp_{qi}",
                              tag=f"cgp_{qi}"),
            "uv": state.tile([128, gsz, 2, N], F16, name=f"uv_{qi}",
                             tag=f"uv_{qi}"),
            "t": state.tile([128, gsz, N], F16, name=f"t_{qi}",
                            tag=f"t_{qi}"),
            "h": state.tile([128, gsz, N], F16, name=f"h_{qi}",
                            tag=f"h_{qi}"),
            "xs": [[state.tile([128, N], F16, name=f"x_{qi}_{ci}_{xi}",
                               tag=f"x_{qi}_{ci}_{xi}") for xi in range(2)]
                   for ci in range(gsz)],
        }
        quads.append(q)

    def step_quad(q, wgx, wgh, tau, exact_tanh, gchunk):
        T, cgp, uv, tq, hq = (q[k] for k in ("T", "cgp", "uv", "t", "h"))
        for ci in range(q["n"]):
            ps = psump.tile([128, 4, 512], F32, name="ps", tag="ps")
            xt = q["xs"][ci][gchunk % 2]
            for b in range(4):
                out = ps[:, b, :N]
                nc.tensor.matmul(out, wgx[:, tau, b, :], xt,
                                 start=True, stop=False)
                nc.tensor.matmul(out, wgh[:, b, :], hq[:, ci, :],
                                 start=False, stop=True)
            # one sigmoid for all four banks (G rows pre-scaled x2),
            # contiguous write into this chain's T slot
            nc.scalar.activation(T[:, ci], ps[:, :, :N], AF.Sigmoid)
        # gp = (G2 - 0.5) * (2/A)  -> tanh(g)/A          [4x]
        nc.vector.tensor_scalar(cgp[:, :, 1], T[:, :, G_BANK], 0.5,
                                2.0 / A_SCALE, ALU.subtract, ALU.mult)
        # (u|v) = (F|I) * (ch|gp), chain-major            [2x]
        nc.vector.tensor_mul(uv, T[:, :, 0:2], cgp)
        # ch' = u + v                                     [2x]
        nc.vector.tensor_add(cgp[:, :, 0], uv[:, :, 0], uv[:, :, 1])
        if exact_tanh:
            nc.scalar.activation(tq, cgp[:, :, 0], AF.Tanh,
                                 scale=float(A_SCALE))
        else:
            nc.vector._custom_dve(tanh5, out=tq, in0=cgp[:, :, 0],
                                  s0=T5_C0, s1=T5_C1, imm2=T5_C2)
        # h = O * tanh(c)                                 [2x]
        nc.vector.tensor_mul(hq, T[:, :, 2], tq)

    assert sum(GROUPS) == N_CHAINS and N_PASS % N_CHAINS == 0
    starts = [sum(GROUPS[:i]) for i in range(len(GROUPS))]
    for g0 in range(0, N_PASS, N_CHAINS):
        passes = [[g0 + starts[qi] + ci for ci in range(GROUPS[qi])]
                  for qi in range(len(GROUPS))]
        # schedule of (phase, chunk) in global step order, for prefetch
        chunk_seq = [("obs", t3) for t3 in range(N_CHUNK_OBS)] + \
                    [("pre", t3) for t3 in range(N_CHUNK_PRE)]
        for qi, q in enumerate(quads):
            for ci in range(q["n"]):
                p = passes[qi][ci]
                nc.sync.dma_start(q["h"][:, ci, :], ins["hT0"][p])
                nc.sync.dma_start(q["cgp"][:, ci, 0, :], ins["cT0"][p])
                nc.sync.dma_start(q["xs"][ci][0], ins["x_obs"][0, p])
                _load_obs_weights()
        if g0 == 0:
            nc.sync.dma_start(wsb["w_gx_pre"],
                              ins["w_gx_pre"].rearrange("t p b m -> p t b m"))
            nc.sync.dma_start(wsb["w_gh_pre"], ins["w_gh_pre"])
        for t in range(T_OBS + T_PRE):
            if t < T_OBS:
                which, tt = "obs", t
            else:
                which, tt = "pre", t - T_OBS
            wgx, wgh = wsb[f"w_gx_{which}"], wsb[f"w_gh_{which}"]
            t3, tau = divmod(tt, XPACK)
            gchunk = (t3 if which == "obs" else N_CHUNK_OBS + t3)
            exact = t in (T_OBS - 1, T_OBS + T_PRE - 1)
            for qi, q in enumerate(quads):
                if t == T_OBS:
                    for ci in range(q["n"]):
                        p = passes[qi][ci]
                        nc.sync.dma_start(outs["hT_obs"][p], q["h"][:, ci, :])
                        nc.sync.dma_start(q["cgp"][:, ci, 0, :],
                                          ins["cT0_pre"][p])
                if tau == 0 and gchunk + 1 < len(chunk_seq):
                    nwhich, nt3 = chunk_seq[gchunk + 1]
                    for ci in range(q["n"]):
                        nc.sync.dma_start(q["xs"][ci][(gchunk + 1) % 2],
                                           ins[f"x_{nwhich}"][nt3,
                                                              passes[qi][ci]])
                step_quad(q, wgx, wgh, tau, exact, gchunk)
        for qi, q in enumerate(quads):
            for ci in range(q["n"]):
                nc.sync.dma_start(outs["hT_pre"][passes[qi][ci]],
                                  q["h"][:, ci, :])

    state.release()
    psump.release()


_CACHED = {}


def _get_program():
    if "nc" in _CACHED:
        return _CACHED["nc"], _CACHED["names"]
    nc = bacc.Bacc("TRN2", target_bir_lowering=False, debug=False,
                   enable_asserts=False, num_devices=N_CORES)
    in_shapes = {
        "x_obs": (N_CHUNK_OBS, N_PASS, 128, N),
        "x_pre": (N_CHUNK_PRE, N_PASS, 128, N),
        "hT0": (N_PASS, 128, N),
        "cT0": (N_PASS, 128, N),
        "cT0_pre": (N_PASS, 128, N),
        "w_gx_obs": (XPACK, 128, 4, 128),
        "w_gh_obs": (128, 4, 128),
        "w_gx_pre": (XPACK, 128, 4, 128),
        "w_gh_pre": (128, 4, 128),
    }
    ins = {
        k: nc.dram_tensor(k, list(s), F16, kind="ExternalInput").ap()
        for k, s in in_shapes.items()
    }
    outs = {
        k: nc.dram_tensor(k, [N_PASS, 128, N], F16, kind="ExternalOutput").ap()
        for k in ("hT_obs", "hT_pre")
    }
    with tile.TileContext(nc) as tc:
        _build_kernel(tc, outs, ins)
    nc.compile()
    _CACHED["nc"] = nc
    _CACHED["names"] = list(in_shapes)
    return nc, _CACHED["names"]


def run(inputs, trace=False, trace_kwargs=None):
    """Run the kernel on 8 cores; returns ((c_out, x_out), BassKernelResults)."""
    nc, _ = _get_program()
    g = lambda k: np.asarray(inputs[k], np.float32)
    wgx_o, wgh_o = _make_weights(g("W_in"), g("b_in"), g("W_ih_obs"),
                                 g("W_hh_obs"), g("b_ih_obs"), g("b_hh_obs"))
    wgx_p, wgh_p = _make_weights(g("W_in"), g("b_in"), g("W_ih_pre"),
                                 g("W_hh_pre"), g("b_ih_pre"), g("b_hh_pre"))
    weights = {"w_gx_obs": wgx_o, "w_gh_obs": wgh_o,
               "w_gx_pre": wgx_p, "w_gh_pre": wgh_p}
    in_maps = [
        _prep_core_inputs(inputs, c * B_C, (c + 1) * B_C, weights)
        for c in range(N_CORES)
    ]
    res = bass_utils.run_bass_kernel_spmd(
        nc, in_maps, core_ids=list(range(N_CORES)), trace=trace,
        **(trace_kwargs or {}))
    hT_obs = np.concatenate(
        [_unshuffle_state(res.results[c]["hT_obs"]) for c in range(N_CORES)],
        axis=1)
    hT_pre = np.concatenate(
        [_unshuffle_state(res.results[c]["hT_pre"]) for c in range(N_CORES)],
        axis=1)
    c_out = hT_obs.reshape(B, H).astype(np.float32)
    x_out = hT_pre.reshape(B, H).astype(np.float32)
    return (c_out, x_out), res


def kernel(**inputs):
    (c_out, x_out), _ = run(inputs)
    return c_out, x_out


# revision 17
# speedup vs baseline: 1.1969x; 1.1969x over previous
"""TRN2 Bass kernel for nn_Encoder (two-phase LSTM over huge batch).

Self-contained: takes the FULL unsharded inputs, shards the batch across
8 NeuronCores (pure data parallel), runs a Bass/Tile kernel per core via
run_bass_kernel_spmd, and reassembles the full outputs.

Device layout (per core, batch B_c = 65536):
  - batch split into 8 chains of 16*512; slice s=0..15 covers 512 columns
    of a chain; SBUF partition p = 8*s + r  <->  (slice s, feature r).
  - one fp16 matmul per gate bank per step: M=128, K=128, block-diagonal
    lhsT (16 8x8 blocks); PSUM accumulates fp32 (x-side + h-side).
  - bank order [F, I, O, G2]; the G rows of the weights are pre-scaled
    by 2 on the host so that tanh(g) = 2*sigmoid(2g) - 1 and ONE Sigmoid
    ACTIVATE covers all four banks (FD=2048) - the scalar engine is the
    eval-count bottleneck, so every saved instruction/eval matters.
  - cell state is stored pre-scaled (ch = c / A): the custom DVE op
    TANH5_ANT (clamp to [-1,1] + odd deg-5 poly, 8 ALU stages, one DVE
    instruction at 1 elem/cycle) evaluates tanh(A*ch) without needing an
    input-scale stage.  tanh work thus moves off the scalar engine for
    all non-output steps; output steps (t=7, 19) use the exact ACT tanh
    (activation scale=A) so the emitted h has table accuracy.
  - DVE (chain-major group tiles, ops batched over a whole group so the
    per-instruction overhead amortizes and packed 2x/4x modes engage):
    gp=(G2-0.5)*(2/A) [tensor_scalar 4x], (u|v)=(F|I)*(ch|gp) in one
    tensor_tensor [2x], ch=u+v [2x], t=TANH5(ch) [1x], h=O*t [2x].
  - x-tiles pack 3 timesteps (row 2*tau+k = x[t0+tau][k]) plus a ones row
    carrying the fused bias; input embedding + biases folded into lhsT;
    x chunks prefetched one chunk ahead on the sync DMA queue.
  - the 8 chains run as 3 groups (3, 3, 2): three pipeline streams whose
    per-group serial path (MMs -> sigmoid -> DVE chain) hides under the
    engine-bound period; PSUM rotates 2 four-bank slots across chains.

  Engine balance per chain-step (512 cols x 16 slices): PE 8 matmuls
  ~1.7us, ACT one FD=2048 sigmoid ~1.9us, DVE ~1.8us - triple-balanced;
  measured 352.6us vs 577.6us baseline, rel err 1.27e-3 (gate 2e-2).
"""

import os
import sys

for _p in ("/opt/trn_rl_repo", "/root/.axon_site/_ro/trn_rl_repo"):
    if os.path.isdir(_p) and _p not in sys.path:
        sys.path.insert(0, _p)
        break

import numpy as np

import concourse.bacc as bacc
import concourse.mybir as mybir
import concourse.tile as tile
from concourse import bass_utils

F32 = mybir.dt.float32
F16 = mybir.dt.float16
AF = mybir.ActivationFunctionType
ALU = mybir.AluOpType

B = 524288
N_CORES = 8
B_C = B // N_CORES
N = 512
SLICES = 16
PASS = SLICES * N
N_PASS = B_C // PASS
T_OBS, T_PRE, IN, H = 8, 12, 2, 8
XPACK = 3
N_CHUNK_OBS = (T_OBS + XPACK - 1) // XPACK
N_CHUNK_PRE = (T_PRE + XPACK - 1) // XPACK
N_CHAINS = 8
# bank order: F, I, O, G (pytorch gate order in the weight rows is
# i, f, g, o).  G rows are scaled x2 (sigmoid trick).
BANK_GATE = [1, 0, 3, 2]
G_BANK = 3

# tanh(A*v) ~= v*(C0 + v^2*(C1 + v^2*C2)) on v in [-1, 1] (distribution-
# weighted fit over the cell-state values this model actually produces).
A_SCALE = 1.5
T5_C0, T5_C1, T5_C2 = 1.48754895, -0.93932143, 0.3667481


# ------------------------------------------------- custom DVE op (TANH5)

_TANH5 = [None]


def _tanh5_ref(in0, in1, s0, s1, imm2):
    v = np.clip(in0, -1.0, 1.0)
    u = v * v
    return v * (s0 + u * (s1 + u * imm2))


def _get_tanh5():
    if _TANH5[0] is not None:
        return _TANH5[0]
    import concourse.dve_ops as dve_ops
    for op in dve_ops.OPS:
        if op.name == "TANH5_ANT":
            _TANH5[0] = op
            return op
    from concourse.dve_spec import Spec, Src0, C0, C1, C2, Zero, One, \
        maxx, minn, sq
    v = maxx(minn(Src0, One), Zero - One)
    u = sq(v)
    spec = Spec(body=v * (C0 + u * (C1 + u * C2)), reference=_tanh5_ref)
    op = dve_ops.DveOp(
        "TANH5_ANT", spec, subdim=False,
        uops_sha={"v3": "71a28649e0d00322", "v4": "c2c1420f4c6160a6"})
    dve_ops.OPS.append(op)
    dve_ops.CUSTOM_DVE_SPECS[op.name] = spec
    dve_ops._SUB_OPCODE_FOR_NAME[op.name] = (
        dve_ops._CUSTOM_DVE_ROW_BASE + len(dve_ops.OPS) - 1)
    assert dve_ops._SUB_OPCODE_FOR_NAME[op.name] < 0x20
    _TANH5[0] = op
    return op


# ---------------------------------------------------------------- host prep

def _make_weights(W_in, b_in, W_ih, W_hh, b_ih, b_hh):
    """lhsT arrays: w_gx [XPACK, 128, 4, 128] (tau,p,bank,m), w_gh [128,4,128].

    Block-diagonal over the 16 slices: one M=128, K=128 matmul per gate bank
    computes that bank for all 16 slices at once.  The G bank rows carry 2x
    the weights/bias (sigmoid trick).
    """
    Wx = (W_ih @ W_in).astype(np.float32)
    bias = (W_ih @ b_in + b_ih + b_hh).astype(np.float32)
    w_gx = np.zeros((XPACK, 128, 4, 128), np.float32)
    w_gh = np.zeros((128, 4, 128), np.float32)
    for b in range(4):
        g = BANK_GATE[b]
        sc = 2.0 if b == G_BANK else 1.0
        for s in range(16):
            for r in range(H):
                col = 8 * s + r
                for tau in range(XPACK):
                    for k in range(IN):
                        w_gx[tau, 8 * s + 2 * tau + k, b, col] = \
                            sc * Wx[g * H + r, k]
                    w_gx[tau, 8 * s + 6, b, col] = sc * bias[g * H + r]
                w_gh[8 * s: 8 * s + H, b, col] = sc * W_hh[g * H + r, :]
    return w_gx.astype(np.float16), w_gh.astype(np.float16)


def _shuffle_state(aT):
    """[8, B_c] -> [N_PASS, 128, N] device layout (p, 8s+r, n)."""
    return np.ascontiguousarray(
        aT.reshape(H, N_PASS, SLICES, N).transpose(1, 2, 0, 3).reshape(
            N_PASS, 128, N).astype(np.float16))


def _unshuffle_state(dev):
    """[N_PASS, 128, N] -> [8, B_c]."""
    return dev.reshape(N_PASS, SLICES, H, N).transpose(2, 0, 1, 3).reshape(
        H, B_C)


def _pack_x(x):
    """[T, 2, B_c] -> [n_chunk, N_PASS, 128, N]: 3 steps + ones row baked."""
    T = x.shape[0]
    n_chunk = (T + XPACK - 1) // XPACK
    out = np.zeros((n_chunk, N_PASS, SLICES, 8, N), np.float32)
    out[:, :, :, 6, :] = 1.0
    for tau in range(XPACK):
        for k in range(IN):
            for t3 in range(n_chunk):
                t = t3 * XPACK + tau
                if t < T:
                    out[t3, :, :, 2 * tau + k, :] = x[t, k].reshape(
                        N_PASS, SLICES, N)
    return np.ascontiguousarray(
        out.reshape(n_chunk, N_PASS, 128, N).astype(np.float16))


def _prep_core_inputs(inputs, lo, hi, weights):
    g = lambda k: np.asarray(inputs[k], np.float32)
    d = {}
    d["x_obs"] = _pack_x(
        np.ascontiguousarray(g("obs_traj_rel")[:, lo:hi, :].transpose(0, 2, 1)))
    d["x_pre"] = _pack_x(
        np.ascontiguousarray(g("pre_traj_rel")[:, lo:hi, :].transpose(0, 2, 1)))
    d["hT0"] = _shuffle_state(np.ascontiguousarray(g("h0")[lo:hi].T))
    d["cT0"] = _shuffle_state(
        np.ascontiguousarray(g("c0")[lo:hi].T) / A_SCALE)
    d["cT0_pre"] = _shuffle_state(
        np.ascontiguousarray(g("c0_pre")[lo:hi].T) / A_SCALE)
    d.update(weights)
    return d


# ------------------------------------------------------------- device build

GROUPS = (3, 3, 2)  # chains per DVE-shared tile group (3 pipeline streams)


def _build_kernel(tc, outs, ins):
    nc = tc.nc
    tanh5 = _get_tanh5()
    state = tc.alloc_tile_pool(name="state", bufs=1)
    psump = tc.alloc_tile_pool(name="psum", bufs=2, space="PSUM")

    # obs-phase weights load first (they gate the very first matmul);
    # pre-phase weights are deferred until after the initial state loads
    # (not needed until t = T_OBS).
    wsb = {}
    for key in ("w_gx_obs", "w_gx_pre"):
        wsb[key] = state.tile([128, XPACK, 4, 128], F16, name=key + "_sb",
                              tag=key)
    for key in ("w_gh_obs", "w_gh_pre"):
        wsb[key] = state.tile([128, 4, 128], F16, name=key + "_sb", tag=key)
    nc.sync.dma_start(wsb["w_gx_obs"],
                      ins["w_gx_obs"].rearrange("t p b m -> p t b m"))
    nc.sync.dma_start(wsb["w_gh_obs"], ins["w_gh_obs"])

    # Chains in a group share one set of wide tiles so every DVE
    # instruction covers group*512 elements (per-instruction overhead
    # amortized, all ops in 2x/4x packed modes).  Three groups give three
    # pipeline streams so the per-group serial path (MM -> sigmoid ->
    # DVE chain) hides under the engine-bound period.  Layouts:
    #   T   [g chains][4 banks F,I,O,G2][N]  <- contiguous-out sigmoids
    #   cgp [g chains][2 slots ch,gp   ][N]
    #   uv  [g chains][2 slots u,v     ][N]
    #   t,h [g chains][N]
    quads = []
    for qi, gsz in enumerate(GROUPS):
        q = {
            "n": gsz,
            "T": state.tile([128, gsz, 4, N], F16, name=f"T_{qi}",
                            tag=f"T_{qi}"),
            "cgp": state.tile([128, gsz, 2, N], F16, name=f"cgp_{qi}",
                              tag=f"cgp_{qi}"),
            "uv": state.tile([128, gsz, 2, N], F16, name=f"uv_{qi}",
                             tag=f"uv_{qi}"),
            "t": state.tile([128, gsz, N], F16, name=f"t_{qi}",
                            tag=f"t_{qi}"),
            "h": state.tile([128, gsz, N], F16, name=f"h_{qi}",
                            tag=f"h_{qi}"),
            "xs": [[state.tile([128, N], F16, name=f"x_{qi}_{ci}_{xi}",
                               tag=f"x_{qi}_{ci}_{xi}") for xi in range(2)]
                   for ci in range(gsz)],
        }
        quads.append(q)

    def step_quad(q, wgx, wgh, tau, exact_tanh, gchunk):
        T, cgp, uv, tq, hq = (q[k] for k in ("T", "cgp", "uv", "t", "h"))
        for ci in range(q["n"]):
            ps = psump.tile([128, 4, 512], F32, name="ps", tag="ps")
            xt = q["xs"][ci][gchunk % 2]
            for b in range(4):
                out = ps[:, b, :N]
                nc.tensor.matmul(out, wgx[:, tau, b, :], xt,
                                 start=True, stop=False)
                nc.tensor.matmul(out, wgh[:, b, :], hq[:, ci, :],
                                 start=False, stop=True)
            # one sigmoid for all four banks (G rows pre-scaled x2),
            # contiguous write into this chain's T slot
            nc.scalar.activation(T[:, ci], ps[:, :, :N], AF.Sigmoid)
        # gp = (G2 - 0.5) * (2/A)  -> tanh(g)/A          [4x]
        nc.vector.tensor_scalar(cgp[:, :, 1], T[:, :, G_BANK], 0.5,
                                2.0 / A_SCALE, ALU.subtract, ALU.mult)
        # (u|v) = (F|I) * (ch|gp), chain-major            [2x]
        nc.vector.tensor_mul(uv, T[:, :, 0:2], cgp)
        # ch' = u + v                                     [2x]
        nc.vector.tensor_add(cgp[:, :, 0], uv[:, :, 0], uv[:, :, 1])
        if exact_tanh:
            nc.scalar.activation(tq, cgp[:, :, 0], AF.Tanh,
                                 scale=float(A_SCALE))
        else:
            nc.vector._custom_dve(tanh5, out=tq, in0=cgp[:, :, 0],
                                  s0=T5_C0, s1=T5_C1, imm2=T5_C2)
        # h = O * tanh(c)                                 [2x]
        nc.vector.tensor_mul(hq, T[:, :, 2], tq)

    assert sum(GROUPS) == N_CHAINS and N_PASS % N_CHAINS == 0
    starts = [sum(GROUPS[:i]) for i in range(len(GROUPS))]
    for g0 in range(0, N_PASS, N_CHAINS):
        passes = [[g0 + starts[qi] + ci for ci in range(GROUPS[qi])]
                  for qi in range(len(GROUPS))]
        # schedule of (phase, chunk) in global step order, for prefetch
        chunk_seq = [("obs", t3) for t3 in range(N_CHUNK_OBS)] + \
                    [("pre", t3) for t3 in range(N_CHUNK_PRE)]
        for qi, q in enumerate(quads):
            for ci in range(q["n"]):
                p = passes[qi][ci]
                nc.sync.dma_start(q["h"][:, ci, :], ins["hT0"][p])
                nc.sync.dma_start(q["cgp"][:, ci, 0, :], ins["cT0"][p])
                nc.sync.dma_start(q["xs"][ci][0], ins["x_obs"][0, p])
        if g0 == 0:
            nc.sync.dma_start(wsb["w_gx_pre"],
                              ins["w_gx_pre"].rearrange("t p b m -> p t b m"))
            nc.sync.dma_start(wsb["w_gh_pre"], ins["w_gh_pre"])
        for t in range(T_OBS + T_PRE):
            if t < T_OBS:
                which, tt = "obs", t
            else:
                which, tt = "pre", t - T_OBS
            wgx, wgh = wsb[f"w_gx_{which}"], wsb[f"w_gh_{which}"]
            t3, tau = divmod(tt, XPACK)
            gchunk = (t3 if which == "obs" else N_CHUNK_OBS + t3)
            exact = t in (T_OBS - 1, T_OBS + T_PRE - 1)
            for qi, q in enumerate(quads):
                if t == T_OBS:
                    for ci in range(q["n"]):
                        p = passes[qi][ci]
                        nc.sync.dma_start(outs["hT_obs"][p], q["h"][:, ci, :])
                        nc.sync.dma_start(q["cgp"][:, ci, 0, :],
                                          ins["cT0_pre"][p])
                if tau == 0 and gchunk + 1 < len(chunk_seq):
                    nwhich, nt3 = chunk_seq[gchunk + 1]
                    for ci in range(q["n"]):
                        nc.sync.dma_start(q["xs"][ci][(gchunk + 1) % 2],
                                           ins[f"x_{nwhich}"][nt3,
                                                              passes[qi][ci]])
                step_quad(q, wgx, wgh, tau, exact, gchunk)
        for qi, q in enumerate(quads):
            for ci in range(q["n"]):
                nc.sync.dma_start(outs["hT_pre"][passes[qi][ci]],
                                  q["h"][:, ci, :])

    state.release()
    psump.release()


_CACHED = {}


def _get_program():
    if "nc" in _CACHED:
        return _CACHED["nc"], _CACHED["names"]
    nc = bacc.Bacc("TRN2", target_bir_lowering=False, debug=False,
                   enable_asserts=False, num_devices=N_CORES)
    in_shapes = {
        "x_obs": (N_CHUNK_OBS, N_PASS, 128, N),
        "x_pre": (N_CHUNK_PRE, N_PASS, 128, N),
        "hT0": (N_PASS, 128, N),
        "cT0": (N_PASS, 128, N),
        "cT0_pre": (N_PASS, 128, N),
        "w_gx_obs": (XPACK, 128, 4, 128),
        "w_gh_obs": (128, 4, 128),
        "w_gx_pre": (XPACK, 128, 4, 128),
        "w_gh_pre": (128, 4, 128),
    }
    ins = {
        k: nc.dram_tensor(k, list(s), F16, kind="ExternalInput").ap()
        for k, s in in_shapes.items()
    }
    outs = {
        k: nc.dram_tensor(k, [N_PASS, 128, N], F16, kind="ExternalOutput").ap()
        for k in ("hT_obs", "hT_pre")
    }
    with tile.TileContext(nc) as tc:
        _build_kernel(tc, outs, ins)
    nc.compile()
    _CACHED["nc"] = nc
    _CACHED["names"] = list(in_shapes)
    return nc, _CACHED["names"]


def run(inputs, trace=False, trace_kwargs=None):
    """Run the kernel on 8 cores; returns ((c_out, x_out), BassKernelResults)."""
    nc, _ = _get_program()
    g = lambda k: np.asarray(inputs[k], np.float32)
    wgx_o, wgh_o = _make_weights(g("W_in"), g("b_in"), g("W_ih_obs"),
                                 g("W_hh_obs"), g("b_ih_obs"), g("b_hh_obs"))
    wgx_p, wgh_p = _make_weights(g("W_in"), g("b_in"), g("W_ih_pre"),
                                 g("W_hh_pre"), g("b_ih_pre"), g("b_hh_pre"))
    weights = {"w_gx_obs": wgx_o, "w_gh_obs": wgh_o,
               "w_gx_pre": wgx_p, "w_gh_pre": wgh_p}
    in_maps = [
        _prep_core_inputs(inputs, c * B_C, (c + 1) * B_C, weights)
        for c in range(N_CORES)
    ]
    res = bass_utils.run_bass_kernel_spmd(
        nc, in_maps, core_ids=list(range(N_CORES)), trace=trace,
        **(trace_kwargs or {}))
    hT_obs = np.concatenate(
        [_unshuffle_state(res.results[c]["hT_obs"]) for c in range(N_CORES)],
        axis=1)
    hT_pre = np.concatenate(
        [_unshuffle_state(res.results[c]["hT_pre"]) for c in range(N_CORES)],
        axis=1)
    c_out = hT_obs.reshape(B, H).astype(np.float32)
    x_out = hT_pre.reshape(B, H).astype(np.float32)
    return (c_out, x_out), res


def kernel(**inputs):
    (c_out, x_out), _ = run(inputs)
    return c_out, x_out
